# revision 7
# baseline (speedup 1.0000x reference)
# GNN edge-apply MLP kernel for Trainium2 (Bass/Tile), 8-core SPMD.
#
# reference semantics:
#   feat = concat(node_feats[src], node_feats[dst], axis=1)      # [E, 2048]
#   h    = relu(feat @ W1 + b1)                                  # [E, 1024]
#   out  = h @ W2 + b2                                           # [E, 1024]
#
# Sharding: edges are split evenly across 8 cores (8192 each); the node
# feature table and MLP weights are replicated.  Inside each core, edges are
# processed in 64 tiles of 128.  Edge e of the core shard maps to
# (p, t) = (e // 64, e % 64): tile t holds edges {p*64 + t}, so the index
# tile loads and the output stores are contiguous per partition.
#
# Per tile:
#   - indirect-DMA gather of 128 src rows and 128 dst rows ([128, 1024] f32)
#   - PE transposes (16 x [128,128]) -> featT chunks (K on partitions)
#   - 32 accumulating matmuls (N=512) -> psum1 [128e, 1024h]
#   - DVE +b1, ACT relu -> h
#   - 8 PE transposes -> hT
#   - 16 matmuls -> psum2 [128e, 1024]
#   - DVE +b2 -> out tile -> HWDGE store
import os
import sys

import numpy as np

for _p in ("/opt/trn_rl_repo",):
    if _p not in sys.path:
        sys.path.insert(0, _p)

N_NODES = 50000
D_NODE = 1024
D_HID = 1024
N_CORES = 8
E_TOTAL = 65536
E_CORE = E_TOTAL // N_CORES
P = 128

# compute dtype for matmuls/transposes: "f32r" (full-rate fp32 storage),
# "f32" (exact, quarter-rate), selected via env for experiments.
COMPUTE = os.environ.get("KERNEL_COMPUTE", "f32r")


def build_nc(e_core=E_CORE, n_nodes=N_NODES, compute=COMPUTE):
    import concourse.bass as bass
    import concourse.mybir as mybir
    import concourse.tile as tile
    from concourse import bacc
    from concourse.masks import make_identity

    f32 = mybir.dt.float32
    i32 = mybir.dt.int32
    # All matmul/transpose operand tiles are allocated in store_dt; the
    # producing ops (DMA cast, DVE copies, ACT relu) perform the rounding
    # the FP32r verifier requires.
    store_dt = {"f32": f32, "f32r": mybir.dt.float32r, "bf16": mybir.dt.bfloat16}[
        compute
    ]

    def mm_view(ap):
        return ap

    T = e_core // P  # edge tiles per core
    KD = (2 * D_NODE) // P  # 16 contraction chunks, layer 1
    KH = D_HID // P  # 8 contraction chunks, layer 2
    NH = 512  # matmul moving free dim (one PSUM bank of fp32)

    nc = bacc.Bacc(None, target_bir_lowering=False)

    nf = nc.dram_tensor("node_feats", [n_nodes, D_NODE], f32, kind="ExternalInput")
    w1 = nc.dram_tensor("W1", [2 * D_NODE, D_HID], f32, kind="ExternalInput")
    w2 = nc.dram_tensor("W2", [D_HID, D_HID], f32, kind="ExternalInput")
    b1 = nc.dram_tensor("b1", [D_HID], f32, kind="ExternalInput")
    b2 = nc.dram_tensor("b2", [D_HID], f32, kind="ExternalInput")
    src = nc.dram_tensor("src", [e_core], i32, kind="ExternalInput")
    dst = nc.dram_tensor("dst", [e_core], i32, kind="ExternalInput")
    out = nc.dram_tensor("out", [e_core, D_HID], f32, kind="ExternalOutput")

    nf_ap = nf.ap()
    out_v = out.ap().rearrange("(p t) h -> p t h", t=T)

    with tile.TileContext(nc) as tc:
        with (
            tc.tile_pool(name="const", bufs=1) as const_pool,
            tc.tile_pool(name="wpool", bufs=1) as wpool,
            tc.tile_pool(name="gather", bufs=3) as gather_pool,
            tc.tile_pool(name="work", bufs=2) as work_pool,
            tc.tile_pool(name="featT", bufs=8) as featT_pool,
            tc.tile_pool(name="hT", bufs=4) as hT_pool,
            tc.tile_pool(name="outp", bufs=3) as out_pool,
            tc.tile_pool(name="pst", bufs=2, space="PSUM") as psT_pool,
            tc.tile_pool(name="ps1", bufs=2, space="PSUM") as ps1_pool,
            tc.tile_pool(name="ps2", bufs=1, space="PSUM") as ps2_pool,
        ):
            # ---- constants / weights ----
            if store_dt == f32:
                ident = const_pool.tile([P, P], f32)
                make_identity(nc, ident[:])
            else:
                # memset/affine_select don't accept f32r/bf16 set values;
                # build in f32 and round via a DVE copy.
                ident_f32 = const_pool.tile([P, P], f32)
                make_identity(nc, ident_f32[:])
                ident = const_pool.tile([P, P], store_dt)
                nc.vector.tensor_copy(ident[:], ident_f32[:])

            idx_src = const_pool.tile([P, T], i32)
            idx_dst = const_pool.tile([P, T], i32)
            nc.sync.dma_start(idx_src[:], src.ap().rearrange("(p t) -> p t", t=T))
            nc.sync.dma_start(idx_dst[:], dst.ap().rearrange("(p t) -> p t", t=T))

            # W1 as [128, KD, 1024]: chunk k rows k*128..k*128+127 of W1
            w1_sb = wpool.tile([P, KD, D_HID], store_dt)
            nc.gpsimd.dma_start(
                w1_sb[:], w1.ap().rearrange("(k p) h -> p k h", p=P)
            )
            w2_sb = wpool.tile([P, KH, D_HID], store_dt)
            nc.gpsimd.dma_start(
                w2_sb[:], w2.ap().rearrange("(k p) h -> p k h", p=P)
            )

            # biases broadcast to all partitions
            b1_bc = const_pool.tile([P, D_HID], f32)
            nc.sync.dma_start(b1_bc[:], b1.ap()[None, :].to_broadcast([P, D_HID]))
            b2_bc = const_pool.tile([P, D_HID], f32)
            nc.sync.dma_start(b2_bc[:], b2.ap()[None, :].to_broadcast([P, D_HID]))

            for t in range(T):
                # ---- gather ----
                src_f = gather_pool.tile([P, D_NODE], store_dt, tag="srcf")
                nc.gpsimd.indirect_dma_start(
                    out=src_f[:],
                    out_offset=None,
                    in_=nf_ap[:],
                    in_offset=bass.IndirectOffsetOnAxis(
                        ap=idx_src[:, t : t + 1], axis=0
                    ),
                )
                dst_f = gather_pool.tile([P, D_NODE], store_dt, tag="dstf")
                nc.gpsimd.indirect_dma_start(
                    out=dst_f[:],
                    out_offset=None,
                    in_=nf_ap[:],
                    in_offset=bass.IndirectOffsetOnAxis(
                        ap=idx_dst[:, t : t + 1], axis=0
                    ),
                )

                # ---- transpose gathered features: 16 [128,128] blocks ----
                # featT group g holds d-chunks 4g..4g+3 side by side in one
                # PSUM bank, then one DVE copy moves them to SBUF.
                featT = []
                for g in range(4):
                    psT = psT_pool.tile([P, 4 * P], store_dt, tag="psT")
                    for j in range(4):
                        k = 4 * g + j
                        blk = (
                            src_f[:, k * P : (k + 1) * P]
                            if k < 8
                            else dst_f[:, (k - 8) * P : (k - 7) * P]
                        )
                        nc.tensor.transpose(
                            mm_view(psT[:, j * P : (j + 1) * P]),
                            mm_view(blk),
                            mm_view(ident[:]),
                        )
                    fT = featT_pool.tile([P, 4 * P], store_dt, tag="featT")
                    nc.vector.tensor_copy(fT[:], psT[:])
                    featT.append(fT)

                # ---- layer 1: psum1[e, h] += featT_k.T-free @ W1[k] ----
                psum1 = ps1_pool.tile([P, D_HID], f32, tag="ps1")
                for k in range(KD):
                    fT = featT[k // 4][:, (k % 4) * P : (k % 4 + 1) * P]
                    for half in range(D_HID // NH):
                        nc.tensor.matmul(
                            psum1[:, half * NH : (half + 1) * NH],
                            mm_view(fT),
                            mm_view(w1_sb[:, k, half * NH : (half + 1) * NH]),
                            start=(k == 0),
                            stop=(k == KD - 1),
                        )

                # ---- bias + relu ----
                h_add = work_pool.tile([P, D_HID], f32, tag="hadd")
                nc.vector.tensor_add(h_add[:], psum1[:], b1_bc[:])
                h_relu = work_pool.tile([P, D_HID], store_dt, tag="hrelu")
                nc.scalar.activation(
                    h_relu[:], h_add[:], mybir.ActivationFunctionType.Relu
                )

                # ---- transpose h: 8 blocks ----
                hT = []
                for g in range(2):
                    psT = psT_pool.tile([P, 4 * P], store_dt, tag="psT")
                    for j in range(4):
                        k = 4 * g + j
                        nc.tensor.transpose(
                            mm_view(psT[:, j * P : (j + 1) * P]),
                            mm_view(h_relu[:, k * P : (k + 1) * P]),
                            mm_view(ident[:]),
                        )
                    hTg = hT_pool.tile([P, 4 * P], store_dt, tag="hT")
                    nc.vector.tensor_copy(hTg[:], psT[:])
                    hT.append(hTg)

                # ---- layer 2 ----
                psum2 = ps2_pool.tile([P, D_HID], f32, tag="ps2")
                for k in range(KH):
                    hTk = hT[k // 4][:, (k % 4) * P : (k % 4 + 1) * P]
                    for half in range(D_HID // NH):
                        nc.tensor.matmul(
                            psum2[:, half * NH : (half + 1) * NH],
                            mm_view(hTk),
                            mm_view(w2_sb[:, k, half * NH : (half + 1) * NH]),
                            start=(k == 0),
                            stop=(k == KH - 1),
                        )

                # ---- bias + store ----
                o_sb = out_pool.tile([P, D_HID], f32, tag="osb")
                nc.vector.tensor_add(o_sb[:], psum2[:], b2_bc[:])
                nc.sync.dma_start(out_v[:, t, :], o_sb[:])

    nc.compile()
    return nc


LAST_RESULTS = None


def kernel(**inputs):
    global LAST_RESULTS
    from concourse.bass_utils import run_bass_kernel_spmd

    node_feats = np.ascontiguousarray(np.asarray(inputs["node_feats"], np.float32))
    W1 = np.ascontiguousarray(np.asarray(inputs["W1"], np.float32))
    W2 = np.ascontiguousarray(np.asarray(inputs["W2"], np.float32))
    b1 = np.ascontiguousarray(np.asarray(inputs["b1"], np.float32))
    b2 = np.ascontiguousarray(np.asarray(inputs["b2"], np.float32))
    src = np.ascontiguousarray(np.asarray(inputs["src"]).astype(np.int32))
    dst = np.ascontiguousarray(np.asarray(inputs["dst"]).astype(np.int32))

    nc = build_nc()

    in_maps = []
    for c in range(N_CORES):
        sl = slice(c * E_CORE, (c + 1) * E_CORE)
        in_maps.append(
            {
                "node_feats": node_feats,
                "W1": W1,
                "W2": W2,
                "b1": b1,
                "b2": b2,
                "src": src[sl],
                "dst": dst[sl],
            }
        )

    trace = bool(int(os.environ.get("KERNEL_TRACE", "0")))
    res = run_bass_kernel_spmd(
        nc, in_maps, core_ids=list(range(N_CORES)), trace=trace
    )
    LAST_RESULTS = res
    return np.concatenate([r["out"] for r in res.results], axis=0)


# revision 13
# speedup vs baseline: 1.4975x; 1.4975x over previous
# GNN edge-apply MLP kernel for Trainium2 (Bass/Tile), 8-core SPMD.
#
# reference semantics:
#   feat = concat(node_feats[src], node_feats[dst], axis=1)      # [E, 2048]
#   h    = relu(feat @ W1 + b1)                                  # [E, 1024]
#   out  = h @ W2 + b2                                           # [E, 1024]
#
# Sharding: edges are split evenly across 8 cores (8192 each); the node
# feature table and MLP weights are replicated.  Inside each core, edges are
# processed in 64 tiles of 128.  Edge e of the core shard maps to
# (p, t) = (e // 64, e % 64): tile t holds edges {p*64 + t}, so the index
# tile loads and the output stores are contiguous per partition.
#
# Per tile:
#   - indirect-DMA gather of 128 src rows and 128 dst rows ([128, 1024] f32)
#   - PE transposes (16 x [128,128]) -> featT chunks (K on partitions)
#   - 32 accumulating matmuls (N=512) -> psum1 [128e, 1024h]
#   - DVE +b1, ACT relu -> h
#   - 8 PE transposes -> hT
#   - 16 matmuls -> psum2 [128e, 1024]
#   - DVE +b2 -> out tile -> HWDGE store
import os
import sys

import numpy as np

for _p in ("/opt/trn_rl_repo",):
    if _p not in sys.path:
        sys.path.insert(0, _p)

N_NODES = 50000
D_NODE = 1024
D_HID = 1024
N_CORES = 8
E_TOTAL = 65536
E_CORE = E_TOTAL // N_CORES
P = 128

# compute dtype for matmuls/transposes: "f32r" (full-rate fp32 storage),
# "f32" (exact, quarter-rate), selected via env for experiments.
COMPUTE = os.environ.get("KERNEL_COMPUTE", "f32r")


def build_nc(e_core=E_CORE, n_nodes=N_NODES, compute=COMPUTE):
    import concourse.bass as bass
    import concourse.mybir as mybir
    import concourse.tile as tile
    from concourse import bacc
    from concourse.masks import make_identity

    f32 = mybir.dt.float32
    i32 = mybir.dt.int32
    # All matmul/transpose operand tiles are allocated in store_dt; the
    # producing ops (DMA cast, DVE copies, ACT relu) perform the rounding
    # the FP32r verifier requires.
    store_dt = {"f32": f32, "f32r": mybir.dt.float32r, "bf16": mybir.dt.bfloat16}[
        compute
    ]

    def mm_view(ap):
        return ap

    T = e_core // P  # edge tiles per core
    KD = (2 * D_NODE) // P  # 16 contraction chunks, layer 1
    KH = D_HID // P  # 8 contraction chunks, layer 2
    NH = 512  # matmul moving free dim (one PSUM bank of fp32)

    nc = bacc.Bacc(None, target_bir_lowering=False)

    nf = nc.dram_tensor("node_feats", [n_nodes, D_NODE], f32, kind="ExternalInput")
    w1 = nc.dram_tensor("W1", [2 * D_NODE, D_HID], f32, kind="ExternalInput")
    w2 = nc.dram_tensor("W2", [D_HID, D_HID], f32, kind="ExternalInput")
    b1 = nc.dram_tensor("b1", [D_HID], f32, kind="ExternalInput")
    b2 = nc.dram_tensor("b2", [D_HID], f32, kind="ExternalInput")
    src = nc.dram_tensor("src", [e_core], i32, kind="ExternalInput")
    dst = nc.dram_tensor("dst", [e_core], i32, kind="ExternalInput")
    out = nc.dram_tensor("out", [e_core, D_HID], f32, kind="ExternalOutput")

    nf_ap = nf.ap()
    out_v = out.ap().rearrange("(p t) h -> p t h", t=T)

    with tile.TileContext(nc) as tc:
        with (
            tc.tile_pool(name="const", bufs=1) as const_pool,
            tc.tile_pool(name="wpool", bufs=1) as wpool,
            tc.tile_pool(name="gather", bufs=4) as gather_pool,
            tc.tile_pool(name="work", bufs=2) as work_pool,
            tc.tile_pool(name="featT", bufs=8) as featT_pool,
            tc.tile_pool(name="hT", bufs=4) as hT_pool,
            tc.tile_pool(name="outp", bufs=3) as out_pool,
            tc.tile_pool(name="pstf", bufs=2, space="PSUM") as psTf_pool,
            tc.tile_pool(name="psth", bufs=1, space="PSUM") as psTh_pool,
            tc.tile_pool(name="psmm", bufs=4, space="PSUM") as psmm_pool,
        ):
            # ---- constants / weights ----
            if store_dt == f32:
                ident = const_pool.tile([P, P], f32)
                make_identity(nc, ident[:])
            else:
                # memset/affine_select don't accept f32r/bf16 set values;
                # build in f32 and round via a DVE copy.
                ident_f32 = const_pool.tile([P, P], f32)
                make_identity(nc, ident_f32[:])
                ident = const_pool.tile([P, P], store_dt)
                nc.vector.tensor_copy(ident[:], ident_f32[:])

            idx_src = const_pool.tile([P, T], i32)
            idx_dst = const_pool.tile([P, T], i32)
            nc.sync.dma_start(idx_src[:], src.ap().rearrange("(p t) -> p t", t=T))
            nc.sync.dma_start(idx_dst[:], dst.ap().rearrange("(p t) -> p t", t=T))

            # W1 as [128, KD, 1024]: chunk k rows k*128..k*128+127 of W1
            w1_sb = wpool.tile([P, KD, D_HID], store_dt)
            nc.gpsimd.dma_start(
                w1_sb[:], w1.ap().rearrange("(k p) h -> p k h", p=P)
            )
            w2_sb = wpool.tile([P, KH, D_HID], store_dt)
            nc.gpsimd.dma_start(
                w2_sb[:], w2.ap().rearrange("(k p) h -> p k h", p=P)
            )

            # biases broadcast to all partitions
            b1_bc = const_pool.tile([P, D_HID], f32)
            nc.sync.dma_start(b1_bc[:], b1.ap()[None, :].to_broadcast([P, D_HID]))
            b2_bc = const_pool.tile([P, D_HID], f32)
            nc.sync.dma_start(b2_bc[:], b2.ap()[None, :].to_broadcast([P, D_HID]))

            # Two-deep software pipeline: the PE stream per iteration is
            # [C1(t) h-transposes | A(t+2) gather-transposes + L1 | C2(t) L2]
            # so every PE->DVE->PE handoff (featT/hT copies, relu) has a full
            # stage of independent PE work to hide under, which also keeps
            # the HAM clock gate warm.  PSUM: psTf 2 + psTh 2 + psmm 4 = 8.
            def stage_A(t):
                """Gathers, feature transposes, layer-1 matmuls -> psum1 halves."""
                src_f = gather_pool.tile([P, D_NODE], store_dt, tag="srcf")
                nc.gpsimd.indirect_dma_start(
                    out=src_f[:],
                    out_offset=None,
                    in_=nf_ap[:],
                    in_offset=bass.IndirectOffsetOnAxis(
                        ap=idx_src[:, t : t + 1], axis=0
                    ),
                )
                dst_f = gather_pool.tile([P, D_NODE], store_dt, tag="dstf")
                nc.gpsimd.indirect_dma_start(
                    out=dst_f[:],
                    out_offset=None,
                    in_=nf_ap[:],
                    in_offset=bass.IndirectOffsetOnAxis(
                        ap=idx_dst[:, t : t + 1], axis=0
                    ),
                )

                featT = []
                for g in range(4):
                    psT = psTf_pool.tile([P, 4 * P], store_dt, tag="psT")
                    for j in range(4):
                        k = 4 * g + j
                        blk = (
                            src_f[:, k * P : (k + 1) * P]
                            if k < 8
                            else dst_f[:, (k - 8) * P : (k - 7) * P]
                        )
                        nc.tensor.transpose(
                            mm_view(psT[:, j * P : (j + 1) * P]),
                            mm_view(blk),
                            mm_view(ident[:]),
                        )
                    fT = featT_pool.tile([P, 4 * P], store_dt, tag="featT")
                    nc.vector.tensor_copy(fT[:], psT[:])
                    featT.append(fT)

                halves = []
                for half in range(D_HID // NH):
                    ps1h = psmm_pool.tile([P, NH], f32, tag="psmm")
                    for k in range(KD):
                        fT = featT[k // 4][:, (k % 4) * P : (k % 4 + 1) * P]
                        nc.tensor.matmul(
                            ps1h[:],
                            mm_view(fT),
                            mm_view(w1_sb[:, k, half * NH : (half + 1) * NH]),
                            start=(k == 0),
                            stop=(k == KD - 1),
                        )
                    halves.append(ps1h)
                return halves

            def stage_B(t, halves):
                """psum1 + b1 -> relu -> h_relu (SBUF)."""
                h_relu = work_pool.tile([P, D_HID], store_dt, tag="hrelu")
                for half, ps1h in enumerate(halves):
                    h_add = work_pool.tile([P, NH], f32, tag="hadd")
                    nc.vector.tensor_add(
                        h_add[:], ps1h[:], b1_bc[:, half * NH : (half + 1) * NH]
                    )
                    nc.scalar.activation(
                        h_relu[:, half * NH : (half + 1) * NH],
                        h_add[:],
                        mybir.ActivationFunctionType.Relu,
                    )
                return h_relu

            def stage_C1(t, h_relu):
                """h transposes into one 2-bank PSUM tile, one DVE copy out."""
                psT = psTh_pool.tile([P, KH * P], store_dt, tag="psTh")
                for k in range(KH):
                    nc.tensor.transpose(
                        mm_view(psT[:, k * P : (k + 1) * P]),
                        mm_view(h_relu[:, k * P : (k + 1) * P]),
                        mm_view(ident[:]),
                    )
                hT = hT_pool.tile([P, KH * P], store_dt, tag="hT")
                nc.vector.tensor_copy(hT[:], psT[:])
                return hT

            def stage_C2(t, hT):
                """Layer-2 matmuls, +b2, store."""
                halves = []
                for half in range(D_HID // NH):
                    ps2h = psmm_pool.tile([P, NH], f32, tag="psmm")
                    for k in range(KH):
                        nc.tensor.matmul(
                            ps2h[:],
                            mm_view(hT[:, k * P : (k + 1) * P]),
                            mm_view(w2_sb[:, k, half * NH : (half + 1) * NH]),
                            start=(k == 0),
                            stop=(k == KH - 1),
                        )
                    halves.append(ps2h)

                o_sb = out_pool.tile([P, D_HID], f32, tag="osb")
                for half, ps2h in enumerate(halves):
                    nc.vector.tensor_add(
                        o_sb[:, half * NH : (half + 1) * NH],
                        ps2h[:],
                        b2_bc[:, half * NH : (half + 1) * NH],
                    )
                nc.sync.dma_start(out_v[:, t, :], o_sb[:])

            # pipeline: A(0); A(1); B(0); then per t: C1(t) B(t+1) A(t+2) C2(t)
            ps1_halves = {0: stage_A(0)}
            if T > 1:
                ps1_halves[1] = stage_A(1)
            h_relus = {0: stage_B(0, ps1_halves.pop(0))}
            for t in range(T):
                hT = stage_C1(t, h_relus.pop(t))
                if t + 1 < T:
                    h_relus[t + 1] = stage_B(t + 1, ps1_halves.pop(t + 1))
                if t + 2 < T:
                    ps1_halves[t + 2] = stage_A(t + 2)
                stage_C2(t, hT)

    nc.compile()
    return nc


LAST_RESULTS = None


def kernel(**inputs):
    global LAST_RESULTS
    from concourse.bass_utils import run_bass_kernel_spmd

    node_feats = np.ascontiguousarray(np.asarray(inputs["node_feats"], np.float32))
    W1 = np.ascontiguousarray(np.asarray(inputs["W1"], np.float32))
    W2 = np.ascontiguousarray(np.asarray(inputs["W2"], np.float32))
    b1 = np.ascontiguousarray(np.asarray(inputs["b1"], np.float32))
    b2 = np.ascontiguousarray(np.asarray(inputs["b2"], np.float32))
    src = np.ascontiguousarray(np.asarray(inputs["src"]).astype(np.int32))
    dst = np.ascontiguousarray(np.asarray(inputs["dst"]).astype(np.int32))

    nc = build_nc()

    in_maps = []
    for c in range(N_CORES):
        sl = slice(c * E_CORE, (c + 1) * E_CORE)
        in_maps.append(
            {
                "node_feats": node_feats,
                "W1": W1,
                "W2": W2,
                "b1": b1,
                "b2": b2,
                "src": src[sl],
                "dst": dst[sl],
            }
        )

    trace = bool(int(os.environ.get("KERNEL_TRACE", "0")))
    res = run_bass_kernel_spmd(
        nc, in_maps, core_ids=list(range(N_CORES)), trace=trace
    )
    LAST_RESULTS = res
    return np.concatenate([r["out"] for r in res.results], axis=0)


# revision 14
# speedup vs baseline: 1.4988x; 1.0009x over previous
# GNN edge-apply MLP kernel for Trainium2 (Bass/Tile), 8-core SPMD.
#
# reference semantics:
#   feat = concat(node_feats[src], node_feats[dst], axis=1)      # [E, 2048]
#   h    = relu(feat @ W1 + b1)                                  # [E, 1024]
#   out  = h @ W2 + b2                                           # [E, 1024]
#
# Sharding: edges are split evenly across 8 cores (8192 each); the node
# feature table and MLP weights are replicated.  Inside each core, edges are
# processed in 64 tiles of 128.  Edge e of the core shard maps to
# (p, t) = (e // 64, e % 64): tile t holds edges {p*64 + t}, so the index
# tile loads and the output stores are contiguous per partition.
#
# Per tile:
#   - indirect-DMA gather of 128 src rows and 128 dst rows ([128, 1024] f32)
#   - PE transposes (16 x [128,128]) -> featT chunks (K on partitions)
#   - 32 accumulating matmuls (N=512) -> psum1 [128e, 1024h]
#   - DVE +b1, ACT relu -> h
#   - 8 PE transposes -> hT
#   - 16 matmuls -> psum2 [128e, 1024]
#   - DVE +b2 -> out tile -> HWDGE store
import os
import sys

import numpy as np

for _p in ("/opt/trn_rl_repo",):
    if _p not in sys.path:
        sys.path.insert(0, _p)

N_NODES = 50000
D_NODE = 1024
D_HID = 1024
N_CORES = 8
E_TOTAL = 65536
E_CORE = E_TOTAL // N_CORES
P = 128

# compute dtype for matmuls/transposes: "f32r" (full-rate fp32 storage),
# "f32" (exact, quarter-rate), selected via env for experiments.
COMPUTE = os.environ.get("KERNEL_COMPUTE", "f32r")


def build_nc(e_core=E_CORE, n_nodes=N_NODES, compute=COMPUTE):
    import concourse.bass as bass
    import concourse.mybir as mybir
    import concourse.tile as tile
    from concourse import bacc
    from concourse.masks import make_identity

    f32 = mybir.dt.float32
    i32 = mybir.dt.int32
    # All matmul/transpose operand tiles are allocated in store_dt; the
    # producing ops (DMA cast, DVE copies, ACT relu) perform the rounding
    # the FP32r verifier requires.
    store_dt = {"f32": f32, "f32r": mybir.dt.float32r, "bf16": mybir.dt.bfloat16}[
        compute
    ]

    def mm_view(ap):
        return ap

    T = e_core // P  # edge tiles per core
    KD = (2 * D_NODE) // P  # 16 contraction chunks, layer 1
    KH = D_HID // P  # 8 contraction chunks, layer 2
    NH = 512  # matmul moving free dim (one PSUM bank of fp32)

    nc = bacc.Bacc(None, target_bir_lowering=False)

    nf = nc.dram_tensor("node_feats", [n_nodes, D_NODE], f32, kind="ExternalInput")
    w1 = nc.dram_tensor("W1", [2 * D_NODE, D_HID], f32, kind="ExternalInput")
    w2 = nc.dram_tensor("W2", [D_HID, D_HID], f32, kind="ExternalInput")
    b1 = nc.dram_tensor("b1", [D_HID], f32, kind="ExternalInput")
    b2 = nc.dram_tensor("b2", [D_HID], f32, kind="ExternalInput")
    src = nc.dram_tensor("src", [e_core], i32, kind="ExternalInput")
    dst = nc.dram_tensor("dst", [e_core], i32, kind="ExternalInput")
    out = nc.dram_tensor("out", [e_core, D_HID], f32, kind="ExternalOutput")

    nf_ap = nf.ap()
    out_v = out.ap().rearrange("(p t) h -> p t h", t=T)

    with tile.TileContext(nc) as tc:
        with (
            tc.tile_pool(name="const", bufs=1) as const_pool,
            tc.tile_pool(name="wpool", bufs=1) as wpool,
            tc.tile_pool(name="gather", bufs=4) as gather_pool,
            tc.tile_pool(name="work", bufs=2) as work_pool,
            tc.tile_pool(name="featT", bufs=8) as featT_pool,
            tc.tile_pool(name="hT", bufs=4) as hT_pool,
            tc.tile_pool(name="outp", bufs=3) as out_pool,
            tc.tile_pool(name="pstf", bufs=2, space="PSUM") as psTf_pool,
            tc.tile_pool(name="psth", bufs=1, space="PSUM") as psTh_pool,
            tc.tile_pool(name="psmm", bufs=4, space="PSUM") as psmm_pool,
        ):
            # ---- constants / weights ----
            if store_dt == f32:
                ident = const_pool.tile([P, P], f32)
                make_identity(nc, ident[:])
            else:
                # memset/affine_select don't accept f32r/bf16 set values;
                # build in f32 and round via a DVE copy.
                ident_f32 = const_pool.tile([P, P], f32)
                make_identity(nc, ident_f32[:])
                ident = const_pool.tile([P, P], store_dt)
                nc.vector.tensor_copy(ident[:], ident_f32[:])

            idx_src = const_pool.tile([P, T], i32)
            idx_dst = const_pool.tile([P, T], i32)
            nc.sync.dma_start(idx_src[:], src.ap().rearrange("(p t) -> p t", t=T))
            nc.sync.dma_start(idx_dst[:], dst.ap().rearrange("(p t) -> p t", t=T))

            # W1 as [128, KD, 1024]: chunk k rows k*128..k*128+127 of W1.
            # Loaded per-chunk so the first layer-1 matmuls aren't gated on
            # the full 12.6MB weight transfer.
            w1_sb = wpool.tile([P, KD, D_HID], store_dt)
            w1_v = w1.ap().rearrange("(k p) h -> p k h", p=P)
            for k in range(KD):
                nc.gpsimd.dma_start(w1_sb[:, k], w1_v[:, k])
            w2_sb = wpool.tile([P, KH, D_HID], store_dt)
            w2_v = w2.ap().rearrange("(k p) h -> p k h", p=P)
            for k in range(KH):
                nc.gpsimd.dma_start(w2_sb[:, k], w2_v[:, k])

            # biases broadcast to all partitions
            b1_bc = const_pool.tile([P, D_HID], f32)
            nc.sync.dma_start(b1_bc[:], b1.ap()[None, :].to_broadcast([P, D_HID]))
            b2_bc = const_pool.tile([P, D_HID], f32)
            nc.sync.dma_start(b2_bc[:], b2.ap()[None, :].to_broadcast([P, D_HID]))

            # Two-deep software pipeline: the PE stream per iteration is
            # [C1(t) h-transposes | A(t+2) gather-transposes + L1 | C2(t) L2]
            # so every PE->DVE->PE handoff (featT/hT copies, relu) has a full
            # stage of independent PE work to hide under, which also keeps
            # the HAM clock gate warm.  PSUM: psTf 2 + psTh 2 + psmm 4 = 8.
            def stage_A(t):
                """Gathers, feature transposes, layer-1 matmuls -> psum1 halves."""
                src_f = gather_pool.tile([P, D_NODE], store_dt, tag="srcf")
                nc.gpsimd.indirect_dma_start(
                    out=src_f[:],
                    out_offset=None,
                    in_=nf_ap[:],
                    in_offset=bass.IndirectOffsetOnAxis(
                        ap=idx_src[:, t : t + 1], axis=0
                    ),
                )
                dst_f = gather_pool.tile([P, D_NODE], store_dt, tag="dstf")
                nc.gpsimd.indirect_dma_start(
                    out=dst_f[:],
                    out_offset=None,
                    in_=nf_ap[:],
                    in_offset=bass.IndirectOffsetOnAxis(
                        ap=idx_dst[:, t : t + 1], axis=0
                    ),
                )

                featT = []
                for g in range(4):
                    psT = psTf_pool.tile([P, 4 * P], store_dt, tag="psT")
                    for j in range(4):
                        k = 4 * g + j
                        blk = (
                            src_f[:, k * P : (k + 1) * P]
                            if k < 8
                            else dst_f[:, (k - 8) * P : (k - 7) * P]
                        )
                        nc.tensor.transpose(
                            mm_view(psT[:, j * P : (j + 1) * P]),
                            mm_view(blk),
                            mm_view(ident[:]),
                        )
                    fT = featT_pool.tile([P, 4 * P], store_dt, tag="featT")
                    nc.vector.tensor_copy(fT[:], psT[:])
                    featT.append(fT)

                halves = []
                for half in range(D_HID // NH):
                    ps1h = psmm_pool.tile([P, NH], f32, tag="psmm")
                    for k in range(KD):
                        fT = featT[k // 4][:, (k % 4) * P : (k % 4 + 1) * P]
                        nc.tensor.matmul(
                            ps1h[:],
                            mm_view(fT),
                            mm_view(w1_sb[:, k, half * NH : (half + 1) * NH]),
                            start=(k == 0),
                            stop=(k == KD - 1),
                        )
                    halves.append(ps1h)
                return halves

            def stage_B(t, halves):
                """psum1 + b1 -> relu -> h_relu (SBUF)."""
                h_relu = work_pool.tile([P, D_HID], store_dt, tag="hrelu")
                for half, ps1h in enumerate(halves):
                    h_add = work_pool.tile([P, NH], f32, tag="hadd")
                    nc.vector.tensor_add(
                        h_add[:], ps1h[:], b1_bc[:, half * NH : (half + 1) * NH]
                    )
                    nc.scalar.activation(
                        h_relu[:, half * NH : (half + 1) * NH],
                        h_add[:],
                        mybir.ActivationFunctionType.Relu,
                    )
                return h_relu

            def stage_C1(t, h_relu):
                """h transposes into one 2-bank PSUM tile, one DVE copy out."""
                psT = psTh_pool.tile([P, KH * P], store_dt, tag="psTh")
                for k in range(KH):
                    nc.tensor.transpose(
                        mm_view(psT[:, k * P : (k + 1) * P]),
                        mm_view(h_relu[:, k * P : (k + 1) * P]),
                        mm_view(ident[:]),
                    )
                hT = hT_pool.tile([P, KH * P], store_dt, tag="hT")
                nc.vector.tensor_copy(hT[:], psT[:])
                return hT

            def stage_C2(t, hT):
                """Layer-2 matmuls, +b2, store."""
                halves = []
                for half in range(D_HID // NH):
                    ps2h = psmm_pool.tile([P, NH], f32, tag="psmm")
                    for k in range(KH):
                        nc.tensor.matmul(
                            ps2h[:],
                            mm_view(hT[:, k * P : (k + 1) * P]),
                            mm_view(w2_sb[:, k, half * NH : (half + 1) * NH]),
                            start=(k == 0),
                            stop=(k == KH - 1),
                        )
                    halves.append(ps2h)

                o_sb = out_pool.tile([P, D_HID], f32, tag="osb")
                for half, ps2h in enumerate(halves):
                    nc.vector.tensor_add(
                        o_sb[:, half * NH : (half + 1) * NH],
                        ps2h[:],
                        b2_bc[:, half * NH : (half + 1) * NH],
                    )
                nc.sync.dma_start(out_v[:, t, :], o_sb[:])

            # pipeline: A(0); A(1); B(0); then per t: C1(t) B(t+1) A(t+2) C2(t)
            ps1_halves = {0: stage_A(0)}
            if T > 1:
                ps1_halves[1] = stage_A(1)
            h_relus = {0: stage_B(0, ps1_halves.pop(0))}
            for t in range(T):
                hT = stage_C1(t, h_relus.pop(t))
                if t + 1 < T:
                    h_relus[t + 1] = stage_B(t + 1, ps1_halves.pop(t + 1))
                if t + 2 < T:
                    ps1_halves[t + 2] = stage_A(t + 2)
                stage_C2(t, hT)

    nc.compile()
    return nc


LAST_RESULTS = None


def kernel(**inputs):
    global LAST_RESULTS
    from concourse.bass_utils import run_bass_kernel_spmd

    node_feats = np.ascontiguousarray(np.asarray(inputs["node_feats"], np.float32))
    W1 = np.ascontiguousarray(np.asarray(inputs["W1"], np.float32))
    W2 = np.ascontiguousarray(np.asarray(inputs["W2"], np.float32))
    b1 = np.ascontiguousarray(np.asarray(inputs["b1"], np.float32))
    b2 = np.ascontiguousarray(np.asarray(inputs["b2"], np.float32))
    src = np.ascontiguousarray(np.asarray(inputs["src"]).astype(np.int32))
    dst = np.ascontiguousarray(np.asarray(inputs["dst"]).astype(np.int32))

    nc = build_nc()

    in_maps = []
    for c in range(N_CORES):
        sl = slice(c * E_CORE, (c + 1) * E_CORE)
        in_maps.append(
            {
                "node_feats": node_feats,
                "W1": W1,
                "W2": W2,
                "b1": b1,
                "b2": b2,
                "src": src[sl],
                "dst": dst[sl],
            }
        )

    trace = bool(int(os.environ.get("KERNEL_TRACE", "0")))
    res = run_bass_kernel_spmd(
        nc, in_maps, core_ids=list(range(N_CORES)), trace=trace
    )
    LAST_RESULTS = res
    return np.concatenate([r["out"] for r in res.results], axis=0)
